# revision 5
# baseline (speedup 1.0000x reference)
"""v7: xy-phase-factored conv, 64x64 PE tiles, 6 matmuls/tile.

v6 was issue-bound: 2880 LDW+MM pairs/core, each MM carrying a
serialized ~26ns semaphore inc -> ~120us.  v7 folds the y-phase into
the contraction dim:

  - x = 4*xb + q (4 phases, as v6), y = 2*yb + s (2 phases, new).
  - contraction rows (per group) = (s, c, q) = 2*8*4 = 64; output
    cols = (t, p, o) = 2*4*8 = 64.  dy = 2*dyd + s - t folds 5 y-taps
    into 3 accumulating matmuls; dx = 4*dl + q - p folds 5 x-taps
    into the lhsT columns with 2 xb deltas -> k = (dyd, dl), NK = 6.
  - free dim = 18 ybo x 27 xbo = 486 (vs 324), chunks of 36 out rows.
  - per core: 2r x 3ch x 16 tiles x 6k = 576 MM pairs (5x fewer).
  - tile_position (64*gh2, 64*h), h = gh2^m -> 4 concurrent quadrants.
  - input DMAs on sync HWDGE ring, output on scalar ring; PSUM
    eviction split between DVE and Activation engines.
"""

import numpy as np
import ml_dtypes

B, C, H, W = 32, 64, 112, 112
O, K, KS = 64, 8, 5
HO, WO = H - KS + 1, W - KS + 1
N_CORES = 8
ROUNDS = 2            # 2 rounds x 2 images = 4 images per core
YB, XB = H // 2, W // 4          # 56, 28
YBO, XBO = 18, WO // 4           # per-chunk out yb, 27
NCH = HO // (2 * YBO)            # 3 chunks of 36 output rows
NFREE = YBO * XBO                # 486
NK = 6                           # 3 dyd x 2 dl

_built = None


def _build_nc():
    import concourse.tile as tile
    import concourse.mybir as mybir
    from concourse import bacc

    nc = bacc.Bacc(None)
    x = nc.dram_tensor("x", [ROUNDS, 2, 4, 128, YB, XB], mybir.dt.bfloat16,
                       kind="ExternalInput")
    w = nc.dram_tensor("w", [128, 4, NK, 64], mybir.dt.bfloat16,
                       kind="ExternalInput")
    bt = nc.dram_tensor("b", [128, 8], mybir.dt.float32, kind="ExternalInput")
    y = nc.dram_tensor("y", [ROUNDS, NCH, 128, 8, NFREE], mybir.dt.bfloat16,
                       kind="ExternalOutput")

    ident = mybir.ActivationFunctionType.Identity

    with tile.TileContext(nc) as tc:
        with (
            tc.tile_pool(name="wp", bufs=1) as wp,
            tc.tile_pool(name="xp", bufs=16) as xp,
            tc.tile_pool(name="op", bufs=2) as op,
            tc.tile_pool(name="bp", bufs=1) as bp,
            tc.tile_pool(name="ps", bufs=8, space="PSUM") as ps,
        ):
            wt = wp.tile([128, 4, NK, 64], mybir.dt.bfloat16)
            bias_sb = bp.tile([128, 8], mybir.dt.float32)
            xts = [[[xp.tile([128, YB, XB], mybir.dt.bfloat16, tag="xt",
                             name=f"xt{r}{m}{gp}")
                     for gp in range(4)] for m in range(2)]
                   for r in range(ROUNDS)]
            ots = [op.tile([128, NCH, 8, NFREE], mybir.dt.bfloat16, tag="ot",
                           name=f"ot{r}") for r in range(ROUNDS)]

            nc.sync.dma_start(wt[:], w[:])
            nc.scalar.dma_start(bias_sb[:], bt[:])
            # r0 in two row-waves (chunk 0 only needs yb < 20); m=0 on the
            # sync HWDGE ring, m=1 on the scalar ring to halve trigger time.
            for b0, b1 in ((0, 20), (20, YB)):
                for m in range(2):
                    eng = nc.sync if m == 0 else nc.scalar
                    for gp in range(4):
                        eng.dma_start(xts[0][m][gp][:, b0:b1, :],
                                      x[0, m, gp][:, b0:b1, :])
            for r in range(1, ROUNDS):
                for m in range(2):
                    for gp in range(4):
                        nc.sync.dma_start(xts[r][m][gp][:], x[r, m, gp])

            # HAM warmup with full-array matmuls during the DMA fill.
            warm = wp.tile([128, 64], mybir.dt.bfloat16, tag="warm")
            nc.vector.memset(warm[:], 0.0)
            wpsum = ps.tile([128, 64], mybir.dt.float32, tag="pt")
            for _ in range(50):
                nc.tensor.matmul(wpsum[:64, :], warm[:], warm[:],
                                 start=True, stop=True)

            for r in range(ROUNDS):
                for ch in range(NCH):
                    y0 = 2 * YBO * ch // 2  # = 18*ch in yb units
                    pts = [ps.tile([128, NFREE], mybir.dt.float32, tag="pt",
                                   name=f"pt{r}_{ch}_{i}") for i in range(8)]
                    for k in range(NK):
                        dyd, dl = divmod(k, 2)
                        for gp in range(4):
                            for m in range(2):
                                for gh2 in range(2):
                                    h = gh2 ^ m
                                    i = 4 * m + gp
                                    nc.tensor.matmul(
                                        pts[i][64 * h: 64 * h + 64, :],
                                        wt[64 * gh2: 64 * gh2 + 64, gp, k, :],
                                        xts[r][m][gp][
                                            64 * gh2: 64 * gh2 + 64,
                                            y0 + dyd: y0 + dyd + YBO,
                                            dl: dl + XBO],
                                        start=(k == 0), stop=(k == NK - 1),
                                        tile_position=(64 * gh2, 64 * h),
                                    )
                    for i in range(8):
                        if i % 2 == 0:
                            nc.vector.tensor_scalar_add(
                                ots[r][:, ch, i, :], pts[i][:],
                                bias_sb[:, i: i + 1])
                        else:
                            nc.scalar.activation(
                                ots[r][:, ch, i, :], pts[i][:], ident,
                                bias=bias_sb[:, i: i + 1])
                    if r == ROUNDS - 1 and ch == NCH - 1:
                        # split the final store so pairs drain as evicted
                        for j in range(4):
                            nc.sync.dma_start(
                                y[r, ch][:, 2 * j: 2 * j + 2, :],
                                ots[r][:, ch, 2 * j: 2 * j + 2, :])
                    else:
                        nc.sync.dma_start(y[r, ch], ots[r][:, ch])
    nc.finalize()
    return nc


def _prep_inputs(X, weight, bias, sel):
    weight = np.asarray(weight, dtype=np.float32)
    sel = np.asarray(sel)
    bias = np.asarray(bias, dtype=np.float32)

    # Dense [O, C, KS, KS] weights via sel scatter.
    wd = np.zeros((O, C, KS, KS), dtype=np.float32)
    for o in range(O):
        for j in range(K):
            wd[o, int(sel[o, j])] += weight[o, j]

    # lhsT: wt[64*gh2 + 32*s + 4*c + q, gp, 2*dyd + dl, 32*t + 8*p + o]
    #     = w_group[g][o, c, 2*dyd + s - t, 4*dl + q - p]
    wt = np.zeros((128, 4, NK, 64), dtype=np.float32)
    for g in range(8):
        gp, gh2 = g % 4, g // 4
        wg = wd[g::8][:, g::8]          # [8 out, 8 in, KS, KS]
        for dyd in range(3):
            for dl in range(2):
                k = 2 * dyd + dl
                for s in range(2):
                    for t in range(2):
                        dy = 2 * dyd + s - t
                        if not 0 <= dy < KS:
                            continue
                        for q in range(4):
                            for p in range(4):
                                dx = 4 * dl + q - p
                                if not 0 <= dx < KS:
                                    continue
                                for c in range(8):
                                    wt[64 * gh2 + 32 * s + 4 * c + q, gp, k,
                                       32 * t + 8 * p: 32 * t + 8 * p + 8] = \
                                        wg[:, c, dy, dx]
    w_host = wt.astype(ml_dtypes.bfloat16)

    # bias_sb[64*h + 32*t + 8*p + o, i] = bias[g + 8*o], g = gp + 4*(h^m)
    bias_sb = np.zeros((128, 8), dtype=np.float32)
    for i in range(8):
        m, gp = divmod(i, 4)
        for h in range(2):
            g = gp + 4 * (h ^ m)
            for t in range(2):
                for p in range(4):
                    r0 = 64 * h + 32 * t + 8 * p
                    bias_sb[r0: r0 + 8, i] = bias[g::8]

    # X [B, C, H, W] -> xF[b, gp, 64*gh2 + 32*s + 4*c + q, yb, xb],
    # channel = g + 8*c, y = 2*yb + s, x = 4*xb + q, g = gp + 4*gh2.
    Xb = np.asarray(X, dtype=np.float32).astype(ml_dtypes.bfloat16)
    arr = Xb.reshape(B, 8, 2, 4, YB, 2, XB, 4)     # b, c, gh2, gp, yb, s, xb, q
    arr = arr.transpose(0, 3, 2, 5, 1, 7, 4, 6)    # b, gp, gh2, s, c, q, yb, xb
    arr = arr.reshape(B, 4, 128, YB, XB)
    xcores = arr.reshape(N_CORES, ROUNDS, 2, 4, 128, YB, XB)

    return [
        {"x": np.ascontiguousarray(xcores[ci]), "w": w_host, "b": bias_sb}
        for ci in range(N_CORES)
    ]


def _postprocess(results):
    outs = []
    for res in results:
        arr = res["y"].reshape(ROUNDS, NCH, 2, 2, 4, 8, 2, 4, YBO, XBO)
        arr = arr.transpose(0, 2, 3, 4, 5, 6, 7, 1, 8, 9)
        #   -> r, h, t, p, o, m, gp, ch, ybo, xbo
        outv = np.zeros((ROUNDS, 2, 8, 8, NCH, YBO, 2, XBO, 4),
                        dtype=ml_dtypes.bfloat16)
        #                r, m, o, g, ch, ybo, t, xbo, p
        for h in range(2):
            for m in range(2):
                gh2 = h ^ m
                sub = arr[:, h, :, :, :, m]   # r, t, p, o, gp, ch, ybo, xbo
                outv[:, m, :, 4 * gh2: 4 * gh2 + 4] = \
                    sub.transpose(0, 3, 4, 5, 6, 1, 7, 2)
        outs.append(outv.reshape(ROUNDS * 2, O, HO, WO).astype(np.float32))
    return np.concatenate(outs, axis=0)


def kernel(X, weight, bias, sel):
    global _built
    from concourse.bass_utils import run_bass_kernel_spmd

    assert X.shape == (B, C, H, W), X.shape
    if _built is None:
        _built = _build_nc()
    in_maps = _prep_inputs(X, weight, bias, sel)
    res = run_bass_kernel_spmd(
        _built, in_maps, core_ids=list(range(N_CORES)), trace=False
    )
    return _postprocess(res.results)
